# revision 1
# baseline (speedup 1.0000x reference)
"""BitNet decoder MLP on 8 Trainium2 NeuronCores (Bass/Tile).

Strategy: data-parallel over batch (512 rows/core). Weights are ternary-quantized
cooperatively (each core quantizes 1/8 of all weight chunks; per-layer
AllReduce for the |W| mean, per-layer AllGather of the quantized bf16 image so
layer 0's weights are available early). All matmul arithmetic is exact:
activations are int8-valued bf16, weights are {-1,0,1} bf16, accumulation fp32
in PSUM. Per-row dequant scales fold into the PSUM-eviction pass on the scalar
engine; LayerNorm+SiLU run as fused scalar-engine passes (sigmoid via the tanh
table for accuracy); rsqrt via Newton on the vector engine; rounding via the
fp32 magic-number trick (round-half-even, matches jnp.round).

Weight images are stored unit-major ("unit" = one weight panel, a
[128, panel_ic*512] block with contiguous per-partition rows) so every big DMA
moves 16KB-contiguous per-partition descriptors.
"""

import numpy as np

import concourse.bass as bass
import concourse.mybir as mybir
import concourse.tile as tile
from concourse import bacc
from concourse.bass_utils import run_bass_kernel_spmd

F32 = mybir.dt.float32
BF16 = mybir.dt.bfloat16
AF = mybir.ActivationFunctionType
OP = mybir.AluOpType

N_CORES = 8
P = 128
OBW = 512            # output block width (one PSUM bank of fp32)
CH_ELS = P * OBW     # elements per weight chunk
MAGIC = 12582912.0   # 1.5 * 2**23: fp32 round-to-nearest-even trick
EPS = 1e-5

FULL_CFG = dict(B=4096, D0=1024, H=4096, OBINS=1000)


def _plan(cfg):
    """Static per-layer plan."""
    B, D0, H, OBINS = cfg["B"], cfg["D0"], cfg["H"], cfg["OBINS"]
    o3_real = 2 * OBINS
    o3_pad = ((o3_real + OBW - 1) // OBW) * OBW
    dims = [
        dict(din=D0, dout=H, dreal=H),
        dict(din=H, dout=H, dreal=H),
        dict(din=H, dout=H, dreal=H),
        dict(din=H, dout=o3_pad, dreal=o3_real),
    ]
    numels = [H * D0, H * H, H * H, o3_real * H]  # real numels for mean|W|
    layers = []
    ch_base = 0
    for li, d in enumerate(dims):
        n_ic = d["din"] // P
        n_ob = d["dout"] // OBW
        n_ch = n_ob * n_ic
        assert n_ch % N_CORES == 0, (li, n_ch)
        panel_ic = min(16, n_ic, max(1, n_ch // N_CORES))
        assert n_ic % panel_ic == 0 and (n_ch // N_CORES) % panel_ic == 0
        n_panels = n_ic // panel_ic
        layers.append(dict(
            li=li, din=d["din"], dout=d["dout"], dreal=d["dreal"],
            n_ic=n_ic, n_ob=n_ob, n_ch=n_ch, per_rank=n_ch // N_CORES,
            panel_ic=panel_ic, n_panels=n_panels,
            numel=numels[li], ch_base=ch_base,
            n_halves=2 if (n_ch // N_CORES) % 2 == 0 and (n_ch // N_CORES) // 2 % panel_ic == 0 else 1,
            ob_w=[min(OBW, d["dreal"] - ob * OBW) for ob in range(n_ob)],
        ))
        ch_base += n_ch
    total_ch = ch_base
    per_rank = total_ch // N_CORES
    b_core = B // N_CORES
    assert b_core % P == 0
    return layers, total_ch, per_rank, b_core // P


def _rsqrt_newton(nc, pool, v, n_iter=3):
    """istd = 1/sqrt(v) for v [128,1] fp32 (v > 0), pure-DVE Newton iteration.

    seed_bits = 0x5f370000 - bits(v)/2 computed in fp32 on aligned int32 views;
    the fp32 mantissa noise on the >2^24 intermediate is irrelevant for a seed.
    """
    seed = pool.tile([P, 1], F32, tag="rs_seed", name="rs_seed")
    seed_i32 = seed[:].bitcast(mybir.dt.int32)
    v_i32 = v.bitcast(mybir.dt.int32)
    nc.vector.tensor_scalar(seed_i32[:], v_i32[:], -0.5,
                            float(0x5F370000), OP.mult, OP.add)
    y = seed
    t1 = pool.tile([P, 1], F32, tag="rs_t1", name="rs_t1")
    t2 = pool.tile([P, 1], F32, tag="rs_t2", name="rs_t2")
    for _ in range(n_iter):
        nc.vector.tensor_tensor(t1[:], y[:], y[:], OP.mult)
        nc.vector.tensor_tensor(t2[:], t1[:], v, OP.mult)
        nc.vector.tensor_scalar(t1[:], t2[:], -0.5, 1.5, OP.mult, OP.add)
        nc.vector.tensor_tensor(y[:], y[:], t1[:], OP.mult)
    return y


def build(cfg):
    layers, total_ch, per_rank, T = _plan(cfg)
    nc = bacc.Bacc("TRN2", target_bir_lowering=False, debug=False,
                   num_devices=N_CORES)

    D0, OBINS = cfg["D0"], cfg["OBINS"]
    b_core = T * P

    xs = nc.dram_tensor("xs", [b_core, D0], F32, kind="ExternalInput")
    # rank's weight chunks, unit-major flat fp32 (see prepare_inputs)
    wsh = nc.dram_tensor("wsh", [per_rank * CH_ELS], F32, kind="ExternalInput")
    mz_out = nc.dram_tensor("mz", [b_core, OBINS], F32, kind="ExternalOutput")
    ii_out = nc.dram_tensor("ii", [b_core, OBINS], F32, kind="ExternalOutput")

    with tile.TileContext(nc) as tc:
        with (
            tc.tile_pool(name="ybig", bufs=4) as ypool,        # 16KB/partition f32
            tc.tile_pool(name="wr", bufs=3) as wrpool,         # prep fp32 runs 8KB
            tc.tile_pool(name="xqT", bufs=5) as xqTpool,       # [128,32,128] bf16
            tc.tile_pool(name="xqT0", bufs=4) as xqT0pool,     # [128,n_ic0,128] bf16
            tc.tile_pool(name="wp", bufs=2) as wpool,          # [128,16,512] bf16
            tc.tile_pool(name="xqn", bufs=2) as xqnpool,       # 8KB/partition bf16
            tc.tile_pool(name="sg", bufs=2) as sgpool,         # [128,512] f32
            tc.tile_pool(name="u", bufs=2) as upool,           # [128,512] f32
            tc.tile_pool(name="outr", bufs=2) as outpool,      # [128,OBINS] f32
            tc.tile_pool(name="small", bufs=1) as small,
            tc.tile_pool(name="psum", bufs=8, space="PSUM") as psum,
            tc.tile_pool(name="dram", bufs=1, space="DRAM") as dram,
        ):
            # ---------------- DRAM scratch (flat, unit-major) ----------------
            stage = []
            image = []
            for L in layers:
                nh = L["n_halves"]
                hs = L["per_rank"] // nh * CH_ELS
                hi = L["n_ch"] // nh * CH_ELS
                stage.append([dram.tile([hs], BF16, tag=f"stage{L['li']}_{h}",
                                        name=f"stage{L['li']}_{h}")
                              for h in range(nh)])
                image.append([dram.tile([hi], BF16, tag=f"image{L['li']}_{h}",
                                        name=f"image{L['li']}_{h}",
                                        addr_space="Shared")
                              for h in range(nh)])
            ar_in = [dram.tile([P, 1], F32, tag=f"ar_in{l}", name=f"ar_in{l}")
                     for l in range(1)]
            ar_out = [dram.tile([P, 1], F32, tag=f"ar_out{l}",
                                name=f"ar_out{l}", addr_space="Shared")
                      for l in range(1)]
            ar_in123 = dram.tile([P, 3], F32, tag="ar_in123", name="ar_in123")
            ar_out123 = dram.tile([P, 3], F32, tag="ar_out123",
                                  name="ar_out123", addr_space="Shared")

            # ---------------- Stage A: input activation quant ----------------
            n_ic0 = layers[0]["n_ic"]
            xqT_cur = []
            am0s = []
            for t in range(T):
                xt = ypool.tile([P, D0], F32, tag="y", name=f"xt{t}")
                nc.sync.dma_start(xt[:], xs[t * P:(t + 1) * P, :])
                am = small.tile([P, 1], F32, tag=f"am0_{t}", name=f"am0_{t}")
                nc.vector.tensor_reduce(am[:], xt[:], mybir.AxisListType.X,
                                        OP.max, apply_absolute_value=True)
                nc.vector.tensor_scalar(am[:], am[:], float(EPS), None, OP.max)
                sc = small.tile([P, 1], F32, tag=f"s0_{t}", name=f"s0_{t}")
                nc.vector.tensor_scalar(sc[:], am[:], 1.0 / 127.0, None, OP.mult)
                nc.vector.reciprocal(sc[:], sc[:])
                xq0 = xqnpool.tile([P, D0], BF16, tag="xqn", name=f"xq0_{t}")
                for ch in range(D0 // OBW):
                    uu = upool.tile([P, OBW], F32, tag="u", name=f"u0_{t}_{ch}")
                    nc.scalar.activation(uu[:], xt[:, ch * OBW:(ch + 1) * OBW],
                                         AF.Copy, bias=MAGIC, scale=sc[:])
                    nc.vector.tensor_scalar(xq0[:, ch * OBW:(ch + 1) * OBW],
                                            uu[:], MAGIC, None, OP.subtract)
                xqT0 = xqT0pool.tile([P, n_ic0, P], BF16, tag="xqT0",
                                     name=f"xqT0_{t}")
                nc.scalar.dma_start_transpose(xqT0[:], xq0[:])
                xqT_cur.append(xqT0)
                am0s.append(am)

            # ------- Stages B/C: scale ARs, quantize, AllGathers -----
            # Critical path: L0 fully first (tiny), then one partial pass +
            # one AllReduce for L1-3, then per-layer quantize + AllGather.
            RUN = 4
            layer_jofs = {}
            jofs = 0
            for L in layers:
                layer_jofs[L["li"]] = jofs
                jofs += L["per_rank"]

            def _abs_pass(li, partial_col, partials):
                L = layers[li]
                pr, jofs = L["per_rank"], layer_jofs[li]
                nrun = 0
                for h in range(0, pr, RUN):
                    rl = min(RUN, pr - h)
                    off = (jofs + h) * CH_ELS
                    wrun = wrpool.tile([P, rl * OBW], F32, tag="wr",
                                       name=f"wrB{li}_{h}")
                    nc.sync.dma_start(
                        wrun[:], wsh[off:off + rl * CH_ELS].rearrange(
                            "(p f) -> p f", p=P))
                    nc.scalar.activation(wrun[:], wrun[:], AF.Abs,
                                         bias=0.0, scale=1.0,
                                         accum_out=partials[:, partial_col + nrun:
                                                            partial_col + nrun + 1])
                    nrun += 1
                return nrun

            def _quant_layer(li, swl):
                L = layers[li]
                pr, jofs = L["per_rank"], layer_jofs[li]
                nh = L["n_halves"]
                prh = pr // nh
                for half in range(nh):
                    for h in range(0, prh, RUN):
                        rl = min(RUN, prh - h)
                        hh = half * prh + h
                        off = (jofs + hh) * CH_ELS
                        wrun = wrpool.tile([P, rl * OBW], F32, tag="wr",
                                           name=f"wrC{li}_{hh}")
                        nc.sync.dma_start(
                            wrun[:], wsh[off:off + rl * CH_ELS].rearrange(
                                "(p f) -> p f", p=P))
                        qrun = outpool.tile([P, rl * OBW], BF16, tag="outr",
                                            name=f"qr{li}_{hh}")
                        for k in range(rl):
                            uu = upool.tile([P, OBW], F32, tag="u",
                                            name=f"uq{li}_{hh}_{k}")
                            nc.scalar.activation(uu[:],
                                                 wrun[:, k * OBW:(k + 1) * OBW],
                                                 AF.Copy, bias=MAGIC, scale=swl[:])
                            vv = sgpool.tile([P, OBW], F32, tag="sg",
                                             name=f"vq{li}_{hh}_{k}")
                            nc.vector.tensor_scalar(vv[:], uu[:], MAGIC, 1.0,
                                                    OP.subtract, OP.min)
                            nc.vector.tensor_scalar(qrun[:, k * OBW:(k + 1) * OBW],
                                                    vv[:], -1.0, None, OP.max)
                        soff = h * CH_ELS
                        nc.sync.dma_start(
                            stage[li][half][soff:soff + rl * CH_ELS].rearrange(
                                "(p f) -> p f", p=P),
                            qrun[:])
                    nc.gpsimd.collective_compute(
                        "AllGather", OP.bypass,
                        ins=[stage[li][half].opt()],
                        outs=[image[li][half].opt()],
                        replica_groups=[list(range(N_CORES))])

            def _scale_post(li, sumcol_ap):
                """From the AllReduced per-partition |W| sums: mean, 1/mean,
                partition-broadcast tiles."""
                L = layers[li]
                mean1 = small.tile([1, 1], F32, tag=f"mean{li}",
                                   name=f"mean{li}")
                nc.gpsimd.tensor_reduce(mean1[:], sumcol_ap,
                                        mybir.AxisListType.C, OP.add)
                nc.vector.tensor_scalar(mean1[:], mean1[:], 1.0 / L["numel"],
                                        float(EPS), OP.mult, OP.max)
                sw1 = small.tile([1, 1], F32, tag=f"sw{li}", name=f"sw{li}")
                nc.vector.reciprocal(sw1[:], mean1[:])
                mwl = small.tile([P, 1], F32, tag=f"mwb{li}", name=f"mwb{li}")
                swl = small.tile([P, 1], F32, tag=f"swb{li}", name=f"swb{li}")
                nc.gpsimd.partition_broadcast(mwl[:], mean1[:])
                nc.gpsimd.partition_broadcast(swl[:], sw1[:])
                return mwl, swl

            n_runs_total = sum((L["per_rank"] + RUN - 1) // RUN for L in layers)
            partials = small.tile([P, n_runs_total], F32, tag="partials",
                                  name="partials")
            mwb = [None] * 4

            # --- L0 chain (short; unblocks the main pass) ---
            nr0 = _abs_pass(0, 0, partials)
            pm0 = small.tile([P, 1], F32, tag="pm0", name="pm0")
            nc.vector.tensor_reduce(pm0[:], partials[:, 0:nr0],
                                    mybir.AxisListType.X, OP.add)
            nc.sync.dma_start(ar_in[0][:], pm0[:])
            nc.gpsimd.collective_compute(
                "AllReduce", OP.add,
                ins=[ar_in[0].opt()], outs=[ar_out[0].opt()],
                replica_groups=[list(range(N_CORES))])
            pms0 = small.tile([P, 1], F32, tag="pms0", name="pms0")
            nc.sync.dma_start(pms0[:], ar_out[0][:])
            mwb[0], swl0 = _scale_post(0, pms0[:])
            _quant_layer(0, swl0)

            # --- L1-3: one partial pass + one AllReduce ---
            col = nr0
            cols = {}
            for li in (1, 2, 3):
                cols[li] = col
                col += _abs_pass(li, col, partials)
            pm123 = small.tile([P, 3], F32, tag="pm123", name="pm123")
            for i, li in enumerate((1, 2, 3)):
                hi = col if li == 3 else cols[li + 1]
                nc.vector.tensor_reduce(pm123[:, i:i + 1],
                                        partials[:, cols[li]:hi],
                                        mybir.AxisListType.X, OP.add)
            nc.sync.dma_start(ar_in123[:], pm123[:])
            nc.gpsimd.collective_compute(
                "AllReduce", OP.add,
                ins=[ar_in123.opt()], outs=[ar_out123.opt()],
                replica_groups=[list(range(N_CORES))])
            pms123 = small.tile([P, 3], F32, tag="pms123", name="pms123")
            nc.sync.dma_start(pms123[:], ar_out123[:])
            for i, li in enumerate((1, 2, 3)):
                mwb[li], swl = _scale_post(li, pms123[:, i:i + 1])
                _quant_layer(li, swl)

            # per-row dequant scale for layer 0
            c_cur = []
            for t in range(T):
                c0 = small.tile([P, 1], F32, tag=f"c0_{t}", name=f"c0_{t}")
                nc.vector.scalar_tensor_tensor(c0[:], am0s[t][:], 1.0 / 127.0,
                                               mwb[0][:], OP.mult, OP.mult)
                c_cur.append(c0)

            # ---------------- Stage D: main pass ----------------
            for L in layers:
                li, n_ic, n_ob = L["li"], L["n_ic"], L["n_ob"]
                panel_ic, n_panels = L["panel_ic"], L["n_panels"]
                dout, dreal = L["dout"], L["dreal"]
                is_last = (li == 3)

                ys = [ypool.tile([P, dreal], F32, tag="y", name=f"y{li}_{t}")
                      for t in range(T)]
                bns = [small.tile([P, n_ob * 6], F32, tag=f"bn{t}",
                                  name=f"bn{li}_{t}")
                       for t in range(T)] if not is_last else None

                for ob in range(n_ob):
                    ow = L["ob_w"][ob]
                    ps = [psum.tile([P, OBW], F32, tag="ps",
                                    name=f"ps{li}_{ob}_{t}") for t in range(T)]
                    for panel in range(n_panels):
                        wp = wpool.tile([P, panel_ic, OBW], BF16, tag="wp",
                                        name=f"wp{li}_{ob}_{panel}")
                        g0 = (ob * n_ic + panel * panel_ic)  # global chunk
                        pr_l = L["per_rank"]
                        prh_l = pr_l // L["n_halves"]
                        rnk, j = divmod(g0, pr_l)
                        half, jl = divmod(j, prh_l)
                        uoff = (rnk * prh_l + jl) * CH_ELS
                        nc.sync.dma_start(
                            wp[:], image[li][half][uoff:uoff + panel_ic * CH_ELS]
                            .rearrange("(p c f) -> p c f", p=P, c=panel_ic))
                        for t in range(T):
                            for cc in range(panel_ic):
                                c = panel * panel_ic + cc
                                nc.tensor.matmul(
                                    ps[t][:], xqT_cur[t][:, c, :],
                                    wp[:, cc, :],
                                    start=(c == 0), stop=(c == n_ic - 1))
                    for t in range(T):
                        dst = ys[t][:, ob * OBW:ob * OBW + ow]
                        if not is_last:
                            nc.scalar.activation(dst, ps[t][:, :ow], AF.Copy,
                                                 bias=0.0, scale=c_cur[t][:])
                            nc.vector.bn_stats(bns[t][:, ob * 6:(ob + 1) * 6], dst)
                        else:
                            nc.scalar.activation(dst, ps[t][:, :ow], AF.Sigmoid,
                                                 bias=0.0, scale=c_cur[t][:])

                if is_last:
                    for t in range(T):
                        mzt = outpool.tile([P, OBINS], F32, tag="outr",
                                           name=f"mzt{t}")
                        nc.vector.tensor_scalar(mzt[:], ys[t][:, 0:OBINS],
                                                float(OBINS - 1), 1.0,
                                                OP.mult, OP.add)
                        nc.scalar.dma_start(mz_out[t * P:(t + 1) * P, :], mzt[:])
                        iit = outpool.tile([P, OBINS], F32, tag="outr",
                                           name=f"iit{t}")
                        nc.vector.tensor_scalar(iit[:], ys[t][:, OBINS:2 * OBINS],
                                                100.0, None, OP.mult)
                        nc.scalar.dma_start(ii_out[t * P:(t + 1) * P, :], iit[:])
                    continue

                # ---- tail: LN + SiLU + act quant + transpose ----
                n_ic_next = layers[li + 1]["n_ic"]
                xqT_next = []
                c_next = []
                for t in range(T):
                    mv = small.tile([P, 2], F32, tag="mv", name=f"mv{li}_{t}")
                    nc.vector.bn_aggr(mv[:], bns[t][:])
                    v = small.tile([P, 1], F32, tag="vvar", name=f"v{li}_{t}")
                    nc.vector.tensor_scalar(v[:], mv[:, 1:2], float(EPS), None,
                                            OP.add)
                    istd = _rsqrt_newton(nc, small, v[:])
                    nmi = small.tile([P, 1], F32, tag="nmi", name=f"nmi{li}_{t}")
                    nc.vector.scalar_tensor_tensor(nmi[:], mv[:, 0:1], -1.0,
                                                   istd[:], OP.mult, OP.mult)
                    # z = (y - mu) * istd, in place
                    nc.scalar.activation(ys[t][:], ys[t][:], AF.Identity,
                                         bias=nmi[:], scale=istd[:])
                    amsl = small.tile([P, 8], F32, tag="amsl",
                                      name=f"amsl{li}_{t}")
                    n_chk = dout // OBW
                    for ch in range(n_chk):
                        sl = ys[t][:, ch * OBW:(ch + 1) * OBW]
                        # sigmoid(z) = 0.5*tanh(0.5*z) + 0.5 (tanh table: 4 ULP)
                        sg = sgpool.tile([P, OBW], F32, tag="sg",
                                         name=f"sg{li}_{t}_{ch}")
                        nc.scalar.activation(sg[:], sl, AF.Tanh,
                                             bias=0.0, scale=0.5)
                        nc.vector.tensor_scalar(sg[:], sg[:], 0.5, 0.5,
                                                OP.mult, OP.add)
                        nc.vector.tensor_tensor(sl, sl, sg[:], OP.mult)
                        nc.vector.tensor_reduce(amsl[:, ch:ch + 1], sl,
                                                mybir.AxisListType.X, OP.max,
                                                apply_absolute_value=True)
                    am = small.tile([P, 1], F32, tag="amn", name=f"am{li}_{t}")
                    nc.vector.tensor_reduce(am[:], amsl[:, :n_chk],
                                            mybir.AxisListType.X, OP.max)
                    nc.vector.tensor_scalar(am[:], am[:], float(EPS), None,
                                            OP.max)
                    sc = small.tile([P, 1], F32, tag="scn", name=f"sc{li}_{t}")
                    nc.vector.tensor_scalar(sc[:], am[:], 1.0 / 127.0, None,
                                            OP.mult)
                    nc.vector.reciprocal(sc[:], sc[:])
                    cn = small.tile([P, 1], F32, tag=f"c{li + 1}_{t}",
                                    name=f"c{li + 1}_{t}")
                    nc.vector.scalar_tensor_tensor(cn[:], am[:], 1.0 / 127.0,
                                                   mwb[li + 1][:],
                                                   OP.mult, OP.mult)
                    c_next.append(cn)
                    xqn = xqnpool.tile([P, dout], BF16, tag="xqn",
                                       name=f"xqn{li}_{t}")
                    for ch in range(n_chk):
                        uu = upool.tile([P, OBW], F32, tag="u",
                                        name=f"ur{li}_{t}_{ch}")
                        nc.scalar.activation(uu[:], ys[t][:, ch * OBW:(ch + 1) * OBW],
                                             AF.Copy, bias=MAGIC, scale=sc[:])
                        nc.vector.tensor_scalar(xqn[:, ch * OBW:(ch + 1) * OBW],
                                                uu[:], MAGIC, None, OP.subtract)
                    xT = xqTpool.tile([P, n_ic_next, P], BF16, tag="xqT",
                                      name=f"xT{li}_{t}")
                    nc.scalar.dma_start_transpose(xT[:], xqn[:])
                    xqT_next.append(xT)
                xqT_cur = xqT_next
                c_cur = c_next

    nc.compile()
    return nc


def prepare_inputs(cfg, x, W0, W1, W2, W3):
    """Host-side sharding: per-core input maps. Weight chunks are shipped
    unit-major: unit u = (layer, ob, panel) is a [128, panel_ic*512] block,
    rows = partitions, contiguous per row; chunk cc of the unit holds
    W_l[ob*512+o, (panel*panel_ic+cc)*128+p] at [p, cc*512+o] (i.e. W^T)."""
    layers, total_ch, per_rank, T = _plan(cfg)
    b_core = T * P
    Ws = [np.asarray(W0), np.asarray(W1), np.asarray(W2), np.asarray(W3)]
    WTs = []
    for L, W in zip(layers, Ws):
        WT = np.zeros((L["din"], L["dout"]), dtype=np.float32)
        WT[:, :L["dreal"]] = W.T
        WTs.append(WT)

    shards = [np.empty(per_rank * CH_ELS, dtype=np.float32)
              for _ in range(N_CORES)]
    for L in layers:
        li, pr = L["li"], L["per_rank"]
        n_ic, panel_ic = L["n_ic"], L["panel_ic"]
        WT = WTs[li]
        for r in range(N_CORES):
            g0 = r * pr
            dst = shards[r]
            for j in range(0, pr, panel_ic):
                g = g0 + j
                ob, ic0 = divmod(g, n_ic)
                assert ic0 % panel_ic == 0
                # unit block [p, cc, o]
                blk = WT[ic0 * P:(ic0 + panel_ic) * P,
                         ob * OBW:(ob + 1) * OBW]          # [panel_ic*128, 512]
                blk = blk.reshape(panel_ic, P, OBW).transpose(1, 0, 2)
                off = (L["ch_base"] // N_CORES + j) * CH_ELS
                dst[off:off + panel_ic * CH_ELS] = blk.reshape(-1)
    x = np.asarray(x, dtype=np.float32)
    in_maps = []
    for r in range(N_CORES):
        in_maps.append(dict(
            xs=np.ascontiguousarray(x[r * b_core:(r + 1) * b_core]),
            wsh=shards[r],
        ))
    return in_maps


_NC_CACHE = {}


def _get_nc(cfg_key):
    if cfg_key not in _NC_CACHE:
        _NC_CACHE[cfg_key] = build(dict(cfg_key))
    return _NC_CACHE[cfg_key]


def run(cfg, x, W0, W1, W2, W3, trace=False):
    layers, total_ch, per_rank, T = _plan(cfg)
    b_core = T * P
    nc = _get_nc(tuple(sorted(cfg.items())))
    in_maps = prepare_inputs(cfg, x, W0, W1, W2, W3)
    res = run_bass_kernel_spmd(nc, in_maps, core_ids=list(range(N_CORES)),
                               trace=trace)
    mz = np.concatenate([res.results[r]["mz"] for r in range(N_CORES)], axis=0)
    ii = np.concatenate([res.results[r]["ii"] for r in range(N_CORES)], axis=0)
    return (mz, ii), res


def kernel(x, W0, W1, W2, W3, g0, b0, g1, b1, g2, b2):
    """Full-input entry point. g/b are identity (ones/zeros) in this problem's
    setup; LayerNorm affine is a no-op and is validated here."""
    for g in (g0, g1, g2):
        assert np.allclose(np.asarray(g), 1.0), "non-identity LN gain unsupported"
    for b in (b0, b1, b2):
        assert np.allclose(np.asarray(b), 0.0), "non-zero LN bias unsupported"
    (mz, ii), _ = run(FULL_CFG, x, W0, W1, W2, W3, trace=False)
    return (mz, ii)



# revision 5
# speedup vs baseline: 4.9885x; 4.9885x over previous
"""BitNet decoder MLP on 8 Trainium2 NeuronCores (Bass/Tile) — v2.

Strategy: data-parallel over batch (512 rows/core). Ternary weight quantization
is cooperative (each core quantizes 1/8 of every layer's chunks), with the
quantized fp8 image ({-1,0,1} exact in fp8e4) delivered by per-layer CHUNKED
AllGathers that are ob-aligned: each AG chunk delivers COMPLETE output blocks
so matmuls chase the AG stream with no PSUM pressure. All matmul arithmetic is
exact: int8-valued bf16 activations (stationary) x ternary fp8 weights
(moving), fp32 PSUM accumulation.

Engine/ring discipline (per-engine instruction order == emission order):
 - sync ring:   x loads, then weight-panel DMAs only (the long trickling
   stream paced by AG completions + pool recycling).
 - scalar ring: x rounds, all weight fp32 read DMAs (issued early, async),
   PSUM evictions, LN/SiLU tail, transposes, output DMAs. qL2/qL3 re-reads
   are issued after the L0 tail so they can't head-of-line-block evictions.
 - vector ring: abs reductions, quant round+clip (3 tensor_scalar passes,
   in-place), bn_stats, tail elementwise, scales.
 - gpsimd ring: AR in/out copies, stage writes, and ALL collective triggers
   (triggers block the ring, so stage writes are interleaved right before
   their own AG trigger).

Collective stream order (FIFO): AR0, AR1(after AG-L0 prep), AG-L0 x2,
AG-L1 c0, AR23, AG-L1 c1..c3, AG-L2 c0..c3, AG-L3 c0..c1.
"""

import numpy as np

import concourse.bass as bass
import concourse.mybir as mybir
import concourse.tile as tile
from concourse import bacc
from concourse.bass_utils import run_bass_kernel_spmd

F32 = mybir.dt.float32
BF16 = mybir.dt.bfloat16
FP8 = mybir.dt.float8e4
AF = mybir.ActivationFunctionType
OP = mybir.AluOpType

N_CORES = 8
P = 128
OBW = 512            # output block width (one PSUM bank of fp32)
CH_ELS = P * OBW     # elements per weight chunk
MAGIC = 12582912.0   # 1.5 * 2**23: fp32 round-to-nearest-even trick
EPS = 1e-5

IMG_DT = BF16        # quantized-image dtype ({-1,0,1} exact in fp8e4)

FULL_CFG = dict(B=4096, D0=1024, H=4096, OBINS=1000)


def _plan(cfg):
    B, D0, H, OBINS = cfg["B"], cfg["D0"], cfg["H"], cfg["OBINS"]
    o3_real = 2 * OBINS
    o3_pad = ((o3_real + OBW - 1) // OBW) * OBW
    dims = [
        dict(din=D0, dout=H, dreal=H),
        dict(din=H, dout=H, dreal=H),
        dict(din=H, dout=H, dreal=H),
        dict(din=H, dout=o3_pad, dreal=o3_real),
    ]
    numels = [H * D0, H * H, H * H, o3_real * H]
    layers = []
    ch_base = 0
    for li, d in enumerate(dims):
        n_ic = d["din"] // P
        n_ob = d["dout"] // OBW
        assert n_ic % N_CORES == 0
        S = n_ic // N_CORES                      # ic chunks per rank
        G = min(4 if li == 0 else 2, n_ob)       # obs per AG chunk
        assert n_ob % G == 0
        n_ag = n_ob // G
        per_rank = n_ag * G * S
        layers.append(dict(
            li=li, din=d["din"], dout=d["dout"], dreal=d["dreal"],
            n_ic=n_ic, n_ob=n_ob, S=S, G=G, n_ag=n_ag,
            per_rank=per_rank, numel=numels[li], ch_base=ch_base,
            ob_w=[min(OBW, d["dreal"] - ob * OBW) for ob in range(n_ob)],
        ))
        ch_base += per_rank
    b_core = B // N_CORES
    assert b_core % P == 0
    return layers, ch_base, b_core // P


def _rsqrt_newton(nc, pool, v, tag, n_iter=3):
    """istd = 1/sqrt(v) for v [128,1] fp32 (v > 0), pure-DVE Newton."""
    seed = pool.tile([P, 1], F32, tag=f"rs_seed{tag}", name=f"rs_seed{tag}")
    seed_i32 = seed[:].bitcast(mybir.dt.int32)
    v_i32 = v.bitcast(mybir.dt.int32)
    nc.vector.tensor_scalar(seed_i32[:], v_i32[:], -0.5,
                            float(0x5F370000), OP.mult, OP.add)
    y = seed
    t1 = pool.tile([P, 1], F32, tag=f"rs_t1{tag}", name=f"rs_t1{tag}")
    t2 = pool.tile([P, 1], F32, tag=f"rs_t2{tag}", name=f"rs_t2{tag}")
    for _ in range(n_iter):
        nc.vector.tensor_tensor(t1[:], y[:], y[:], OP.mult)
        nc.vector.tensor_tensor(t2[:], t1[:], v, OP.mult)
        nc.vector.tensor_scalar(t1[:], t2[:], -0.5, 1.5, OP.mult, OP.add)
        nc.vector.tensor_tensor(y[:], y[:], t1[:], OP.mult)
    return y


def build(cfg):
    layers, total_ch, T = _plan(cfg)
    nc = bacc.Bacc("TRN2", target_bir_lowering=False, debug=False,
                   num_devices=N_CORES)

    D0, OBINS = cfg["D0"], cfg["OBINS"]
    b_core = T * P
    n_ic0 = layers[0]["n_ic"]
    YW = max(D0, max(L["dout"] for L in layers))

    xs = nc.dram_tensor("xs", [b_core, D0], F32, kind="ExternalInput")
    wsh = nc.dram_tensor("wsh", [total_ch * CH_ELS], F32, kind="ExternalInput")
    mz_out = nc.dram_tensor("mz", [b_core, OBINS], F32, kind="ExternalOutput")
    ii_out = nc.dram_tensor("ii", [b_core, OBINS], F32, kind="ExternalOutput")

    RUN = 4  # chunks per abs-pass read

    with tile.TileContext(nc) as tc:
        with (
            tc.tile_pool(name="y", bufs=4) as ypool,          # [128,4096] f32
            tc.tile_pool(name="xqT", bufs=4) as xqTpool,      # [128,n_ic,128] bf16
            tc.tile_pool(name="xqT0", bufs=4) as xqT0pool,    # [128,n_ic0,128] bf16
            tc.tile_pool(name="wp", bufs=16) as wpool,        # [128,2048] fp8
            tc.tile_pool(name="xqn", bufs=2) as xqnpool,      # [128,4096] bf16
            tc.tile_pool(name="wr", bufs=2) as wrpool,        # [128,2048] f32
            tc.tile_pool(name="qq", bufs=2) as qqpool,        # [128,2048] fp8
            tc.tile_pool(name="sg", bufs=2) as sgpool,        # [128,1024] f32
            tc.tile_pool(name="u", bufs=2) as upool,          # [128,1024] f32
            tc.tile_pool(name="outr", bufs=2) as outpool,     # [128,OBINS] f32
            tc.tile_pool(name="small", bufs=1) as small,
            tc.tile_pool(name="psum", bufs=8, space="PSUM") as psum,
            tc.tile_pool(name="dram", bufs=1, space="DRAM") as dram,
        ):
            # ---------------- DRAM scratch ----------------
            stage = []   # stage[li][k]: per-rank quantized chunk (fp8)
            image = []   # image[li][k]: gathered (8x) chunk (fp8)
            for L in layers:
                li, G, S = L["li"], L["G"], L["S"]
                sz = G * S * CH_ELS
                stage.append([dram.tile([sz], IMG_DT, tag=f"st{li}_{k}",
                                        name=f"st{li}_{k}")
                              for k in range(L["n_ag"])])
                image.append([dram.tile([N_CORES * sz], IMG_DT,
                                        tag=f"im{li}_{k}", name=f"im{li}_{k}",
                                        addr_space="Shared")
                              for k in range(L["n_ag"])])
            ar_in0 = dram.tile([P, 1], F32, tag="ar_in0", name="ar_in0")
            ar_out0 = dram.tile([P, 1], F32, tag="ar_out0", name="ar_out0",
                                addr_space="Shared")
            ar_in1 = dram.tile([P, 1], F32, tag="ar_in1", name="ar_in1")
            ar_out1 = dram.tile([P, 1], F32, tag="ar_out1", name="ar_out1",
                                addr_space="Shared")
            ar_in23 = dram.tile([P, 2], F32, tag="ar_in23", name="ar_in23")
            ar_out23 = dram.tile([P, 2], F32, tag="ar_out23", name="ar_out23",
                                 addr_space="Shared")

            rg = [list(range(N_CORES))]

            # =========== 1. x load + act quant + transpose ===========
            xqT_cur = []
            am0s = []
            for t in range(T):
                xt_full = ypool.tile([P, YW], F32, tag="y", name=f"xt{t}")
                xt = xt_full[:, :D0]
                nc.sync.dma_start(xt, xs[t * P:(t + 1) * P, :])
                am = small.tile([P, 1], F32, tag=f"am0_{t}", name=f"am0_{t}")
                nc.vector.tensor_reduce(am[:], xt, mybir.AxisListType.X,
                                        OP.max, apply_absolute_value=True)
                nc.vector.tensor_scalar(am[:], am[:], float(EPS), None, OP.max)
                sc = small.tile([P, 1], F32, tag=f"s0_{t}", name=f"s0_{t}")
                nc.vector.tensor_scalar(sc[:], am[:], 1.0 / 127.0, None,
                                        OP.mult)
                nc.vector.reciprocal(sc[:], sc[:])
                xq0 = xqnpool.tile([P, YW], BF16, tag="xqn", name=f"xq0_{t}")
                for ch in range(D0 // OBW):
                    uu = upool.tile([P, min(OBW, 1024)], F32, tag="u",
                                    name=f"u0_{t}_{ch}")
                    nc.scalar.activation(uu[:, :OBW],
                                         xt[:, ch * OBW:(ch + 1) * OBW],
                                         AF.Copy, bias=MAGIC, scale=sc[:])
                    nc.vector.tensor_scalar(xq0[:, ch * OBW:(ch + 1) * OBW],
                                            uu[:, :OBW], MAGIC, None,
                                            OP.subtract)
                xqT0 = xqT0pool.tile([P, n_ic0, P], BF16, tag="xqT0",
                                     name=f"xqT0_{t}")
                nc.scalar.dma_start_transpose(xqT0[:], xq0[:, :D0])
                xqT_cur.append(xqT0)
                am0s.append(am)

            # =========== helpers ===========
            partials = small.tile([P, 32], F32, tag="partials",
                                  name="partials")
            pcol = [0]

            def _abs_pass(li):
                """Read the rank's fp32 chunks (scalar ring) and abs-sum
                (vector). Returns the partials column range."""
                L = layers[li]
                pr = L["per_rank"]
                base = L["ch_base"]
                c0 = pcol[0]
                for h in range(0, pr, RUN):
                    rl = min(RUN, pr - h)
                    off = (base + h) * CH_ELS
                    wrun = wrpool.tile([P, RUN * OBW], F32, tag="wr",
                                       name=f"wa{li}_{h}")
                    nc.scalar.dma_start(
                        wrun[:, :rl * OBW],
                        wsh[off:off + rl * CH_ELS].rearrange(
                            "(p f) -> p f", p=P))
                    nc.vector.tensor_reduce(
                        partials[:, pcol[0]:pcol[0] + 1], wrun[:, :rl * OBW],
                        mybir.AxisListType.X, OP.add,
                        apply_absolute_value=True)
                    pcol[0] += 1
                return c0, pcol[0]

            def _scale_post(li, sumcol_ap):
                L = layers[li]
                mean1 = small.tile([1, 1], F32, tag=f"mean{li}",
                                   name=f"mean{li}")
                nc.gpsimd.tensor_reduce(mean1[:], sumcol_ap,
                                        mybir.AxisListType.C, OP.add)
                nc.vector.tensor_scalar(mean1[:], mean1[:], 1.0 / L["numel"],
                                        float(EPS), OP.mult, OP.max)
                sw1 = small.tile([1, 1], F32, tag=f"sw{li}", name=f"sw{li}")
                nc.vector.reciprocal(sw1[:], mean1[:])
                mwl = small.tile([P, 1], F32, tag=f"mwb{li}", name=f"mwb{li}")
                swl = small.tile([P, 1], F32, tag=f"swb{li}", name=f"swb{li}")
                nc.gpsimd.partition_broadcast(mwl[:], mean1[:])
                nc.gpsimd.partition_broadcast(swl[:], sw1[:])
                return mwl, swl

            def _quant_chunk(li, k, swl):
                """Re-read chunk k's fp32 (scalar ring), round+clip to fp8
                (vector, in-place), stage write (gpsimd)."""
                L = layers[li]
                G, S = L["G"], L["S"]
                base = L["ch_base"] + k * G * S
                for g in range(G):
                    off = (base + g * S) * CH_ELS
                    w = S * OBW
                    wrun = wrpool.tile([P, RUN * OBW], F32, tag="wr",
                                       name=f"wq{li}_{k}_{g}")
                    nc.scalar.dma_start(
                        wrun[:, :w],
                        wsh[off:off + S * CH_ELS].rearrange(
                            "(p f) -> p f", p=P))
                    nc.vector.tensor_scalar(wrun[:, :w], wrun[:, :w], swl[:],
                                            MAGIC, OP.mult, OP.add)
                    nc.vector.tensor_scalar(wrun[:, :w], wrun[:, :w], MAGIC,
                                            1.0, OP.subtract, OP.min)
                    q = qqpool.tile([P, RUN * OBW], IMG_DT, tag="qq",
                                    name=f"q{li}_{k}_{g}")
                    nc.vector.tensor_scalar(q[:, :w], wrun[:, :w], -1.0, None,
                                            OP.max)
                    soff = g * S * CH_ELS
                    nc.gpsimd.dma_start(
                        stage[li][k][soff:soff + S * CH_ELS].rearrange(
                            "(p f) -> p f", p=P),
                        q[:, :w])

            def _ag(li, k):
                nc.gpsimd.collective_compute(
                    "AllGather", OP.bypass,
                    ins=[stage[li][k].opt()], outs=[image[li][k].opt()],
                    replica_groups=rg)

            # =========== 2. abs L0 + AR0 + post ===========
            a0, a1 = _abs_pass(0)
            pm0 = small.tile([P, 1], F32, tag="pm0", name="pm0")
            nc.vector.tensor_reduce(pm0[:], partials[:, a0:a1],
                                    mybir.AxisListType.X, OP.add)
            nc.gpsimd.dma_start(ar_in0[:], pm0[:])
            nc.gpsimd.collective_compute("AllReduce", OP.add,
                                         ins=[ar_in0.opt()],
                                         outs=[ar_out0.opt()],
                                         replica_groups=rg)
            pms0 = small.tile([P, 1], F32, tag="pms0", name="pms0")
            nc.gpsimd.dma_start(pms0[:], ar_out0[:])
            mwb0, swl0 = _scale_post(0, pms0[:])

            # c0 scales (vector; needed by first L0 evictions)
            c_cur = []
            for t in range(T):
                c0t = small.tile([P, 1], F32, tag=f"c0_{t}", name=f"c0_{t}")
                nc.vector.scalar_tensor_tensor(c0t[:], am0s[t][:], 1.0 / 127.0,
                                               mwb0[:], OP.mult, OP.mult)
                c_cur.append(c0t)

            # =========== 3. quant L0 + AG L0 ===========
            for k in range(layers[0]["n_ag"]):
                _quant_chunk(0, k, swl0)
                _ag(0, k)

            # =========== 4. abs L1 + AR1 + post + quant c0 + AG c0 ===========
            b0, b1 = _abs_pass(1)
            pm1 = small.tile([P, 1], F32, tag="pm1", name="pm1")
            nc.vector.tensor_reduce(pm1[:], partials[:, b0:b1],
                                    mybir.AxisListType.X, OP.add)
            nc.gpsimd.dma_start(ar_in1[:], pm1[:])
            nc.gpsimd.collective_compute("AllReduce", OP.add,
                                         ins=[ar_in1.opt()],
                                         outs=[ar_out1.opt()],
                                         replica_groups=rg)
            pms1 = small.tile([P, 1], F32, tag="pms1", name="pms1")
            nc.gpsimd.dma_start(pms1[:], ar_out1[:])
            mwb1, swl1 = _scale_post(1, pms1[:])
            _quant_chunk(1, 0, swl1)
            _ag(1, 0)

            # =========== 5. abs L2/L3 + AR23 (cc slot before AG-L1 c1+) =====
            c0r, c1r = _abs_pass(2)
            d0r, d1r = _abs_pass(3)
            pm23 = small.tile([P, 2], F32, tag="pm23", name="pm23")
            nc.vector.tensor_reduce(pm23[:, 0:1], partials[:, c0r:c1r],
                                    mybir.AxisListType.X, OP.add)
            nc.vector.tensor_reduce(pm23[:, 1:2], partials[:, d0r:d1r],
                                    mybir.AxisListType.X, OP.add)
            nc.gpsimd.dma_start(ar_in23[:], pm23[:])
            nc.gpsimd.collective_compute("AllReduce", OP.add,
                                         ins=[ar_in23.opt()],
                                         outs=[ar_out23.opt()],
                                         replica_groups=rg)
            pms23 = small.tile([P, 2], F32, tag="pms23", name="pms23")
            nc.gpsimd.dma_start(pms23[:], ar_out23[:])
            mwb2, swl2 = _scale_post(2, pms23[:, 0:1])
            mwb3, swl3 = _scale_post(3, pms23[:, 1:2])
            mwb = [mwb0, mwb1, mwb2, mwb3]

            # =========== 6. quant L1 c1..c3 + AGs ===========
            for k in range(1, layers[1]["n_ag"]):
                _quant_chunk(1, k, swl1)
                _ag(1, k)

            # =========== 7. main pass ===========
            def _main_layer(li, xqT, c_in):
                """Matmuls + evictions for layer li. Returns ys, bns."""
                L = layers[li]
                G, S, n_ob = L["G"], L["S"], L["n_ob"]
                dreal = L["dreal"]
                is_last = (li == 3)
                blk = G * S * CH_ELS  # per-rank bytes-elems in one AG chunk
                ys = [ypool.tile([P, YW], F32, tag="y",
                                 name=f"y{li}_{t}")
                      for t in range(T)]
                bns = None
                if not is_last:
                    bns = [small.tile([P, n_ob * 6], F32, tag=f"bn{li}_{t}",
                                      name=f"bn{li}_{t}") for t in range(T)]

                if li == 0:
                    # quad-cached panels: one DMA per (AG chunk, rseg),
                    # wp [128, G, 512] reused across the chunk's G obs.
                    for k in range(L["n_ag"]):
                        wps = []
                        for rseg in range(N_CORES):
                            wp = wpool.tile([P, 4 * OBW], IMG_DT, tag="wp",
                                            name=f"wp0_{k}_{rseg}")
                            off = rseg * blk
                            nc.sync.dma_start(
                                wp[:, :G * OBW].rearrange(
                                    "p (g f) -> p g f", g=G),
                                image[0][k][off:off + blk].rearrange(
                                    "(g p f) -> p g f", p=P, g=G))
                            wps.append(wp)
                        for g in range(G):
                            ob = k * G + g
                            for t in range(T):
                                ps = psum.tile([P, OBW], F32, tag="ps",
                                               name=f"ps0_{ob}_{t}")
                                for rseg in range(N_CORES):
                                    nc.tensor.matmul(
                                        ps[:], xqT[t][:, rseg, :],
                                        wps[rseg][:, g * OBW:(g + 1) * OBW],
                                        start=(rseg == 0),
                                        stop=(rseg == N_CORES - 1))
                                dst = ys[t][:, ob * OBW:ob * OBW + OBW]
                                nc.scalar.activation(dst, ps[:], AF.Copy,
                                                     bias=0.0, scale=c_in[t][:])
                                nc.vector.bn_stats(
                                    bns[t][:, ob * 6:(ob + 1) * 6], dst)
                else:
                    for ob in range(n_ob):
                        k, g = divmod(ob, G)
                        ow = L["ob_w"][ob]
                        wps = []
                        for rseg in range(N_CORES):
                            wp = wpool.tile([P, 4 * OBW], IMG_DT, tag="wp",
                                            name=f"wp{li}_{ob}_{rseg}")
                            off = (rseg * G + g) * S * CH_ELS
                            nc.sync.dma_start(
                                wp[:, :S * OBW],
                                image[li][k][off:off + S * CH_ELS].rearrange(
                                    "(p f) -> p f", p=P))
                            wps.append(wp)
                        for t in range(T):
                            ps = psum.tile([P, OBW], F32, tag="ps",
                                           name=f"ps{li}_{ob}_{t}")
                            for rseg in range(N_CORES):
                                for s in range(S):
                                    nc.tensor.matmul(
                                        ps[:], xqT[t][:, rseg * S + s, :],
                                        wps[rseg][:, s * OBW:(s + 1) * OBW],
                                        start=(rseg == 0 and s == 0),
                                        stop=(rseg == N_CORES - 1
                                              and s == S - 1))
                            dst = ys[t][:, ob * OBW:ob * OBW + ow]
                            if not is_last:
                                nc.scalar.activation(dst, ps[:, :ow], AF.Copy,
                                                     bias=0.0,
                                                     scale=c_in[t][:])
                                nc.vector.bn_stats(
                                    bns[t][:, ob * 6:(ob + 1) * 6], dst)
                            else:
                                nc.scalar.activation(dst, ps[:, :ow],
                                                     AF.Sigmoid, bias=0.0,
                                                     scale=c_in[t][:])
                return ys, bns

            def _tail_layer(li, ys, bns):
                """LN + SiLU + act quant + transpose; returns (xqT_next,
                c_next)."""
                L = layers[li]
                dout = L["dout"]
                n_ob = L["n_ob"]
                n_ic_next = layers[li + 1]["n_ic"]
                CW = 1024
                n_cw = dout // CW
                xqT_next, c_next = [], []
                for t in range(T):
                    mv = small.tile([P, 2], F32, tag="mv", name=f"mv{li}_{t}")
                    nc.vector.bn_aggr(mv[:], bns[t][:, :n_ob * 6])
                    v = small.tile([P, 1], F32, tag="vvar", name=f"v{li}_{t}")
                    nc.vector.tensor_scalar(v[:], mv[:, 1:2], float(EPS),
                                            None, OP.add)
                    istd = _rsqrt_newton(nc, small, v[:], tag="")
                    # z2 = 0.5*(y-mu)*istd  (so silu = (tanh(z2)+1)*z2)
                    sc2 = small.tile([P, 1], F32, tag="sc2",
                                     name=f"sc2{li}_{t}")
                    nc.vector.tensor_scalar(sc2[:], istd[:], 0.5, None,
                                            OP.mult)
                    nmi = small.tile([P, 1], F32, tag="nmi",
                                     name=f"nmi{li}_{t}")
                    nc.vector.scalar_tensor_tensor(nmi[:], mv[:, 0:1], -1.0,
                                                   sc2[:], OP.mult, OP.mult)
                    yt = ys[t][:, :dout]
                    nc.scalar.activation(yt, yt, AF.Identity,
                                         bias=nmi[:], scale=sc2[:])
                    amsl = small.tile([P, 4], F32, tag="amsl",
                                      name=f"amsl{li}_{t}")
                    for ch in range(n_cw):
                        sl = ys[t][:, ch * CW:(ch + 1) * CW]
                        sgt = sgpool.tile([P, CW], F32, tag="sg",
                                          name=f"sg{li}_{t}_{ch}")
                        nc.scalar.activation(sgt[:], sl, AF.Tanh,
                                             bias=0.0, scale=1.0)
                        # silu = (tanh(z2)+1)*z2, in place
                        nc.vector.scalar_tensor_tensor(sl, sgt[:], 1.0, sl,
                                                       OP.add, OP.mult)
                        nc.vector.tensor_reduce(amsl[:, ch:ch + 1], sl,
                                                mybir.AxisListType.X, OP.max,
                                                apply_absolute_value=True)
                    am = small.tile([P, 1], F32, tag="amn", name=f"am{li}_{t}")
                    nc.vector.tensor_reduce(am[:], amsl[:, :n_cw],
                                            mybir.AxisListType.X, OP.max)
                    nc.vector.tensor_scalar(am[:], am[:], float(EPS), None,
                                            OP.max)
                    sc = small.tile([P, 1], F32, tag="scn", name=f"sc{li}_{t}")
                    nc.vector.tensor_scalar(sc[:], am[:], 1.0 / 127.0, None,
                                            OP.mult)
                    nc.vector.reciprocal(sc[:], sc[:])
                    cn = small.tile([P, 1], F32, tag=f"c{li + 1}_{t}",
                                    name=f"c{li + 1}_{t}")
                    nc.vector.scalar_tensor_tensor(cn[:], am[:], 1.0 / 127.0,
                                                   mwb[li + 1][:],
                                                   OP.mult, OP.mult)
                    c_next.append(cn)
                    xqn = xqnpool.tile([P, YW], BF16, tag="xqn",
                                       name=f"xqn{li}_{t}")
                    for ch in range(n_cw):
                        uu = upool.tile([P, CW], F32, tag="u",
                                        name=f"ur{li}_{t}_{ch}")
                        nc.scalar.activation(uu[:],
                                             ys[t][:, ch * CW:(ch + 1) * CW],
                                             AF.Copy, bias=MAGIC, scale=sc[:])
                        nc.vector.tensor_scalar(xqn[:, ch * CW:(ch + 1) * CW],
                                                uu[:], MAGIC, None,
                                                OP.subtract)
                    xT = xqTpool.tile([P, n_ic_next, P], BF16, tag="xqT",
                                      name=f"xT{li}_{t}")
                    nc.scalar.dma_start_transpose(xT[:], xqn[:, :dout])
                    xqT_next.append(xT)
                return xqT_next, c_next

            # L0
            ys, bns = _main_layer(0, xqT_cur, c_cur)
            xqT_cur, c_cur = _tail_layer(0, ys, bns)

            # qL2/qL3 + AGs (emitted after L0 tail: scalar-ring reads can't
            # block L0 evictions; gpsimd stage writes sit right before their
            # own triggers)
            for k in range(layers[2]["n_ag"]):
                _quant_chunk(2, k, swl2)
                _ag(2, k)
            for k in range(layers[3]["n_ag"]):
                _quant_chunk(3, k, swl3)
                _ag(3, k)

            # L1, L2
            ys, bns = _main_layer(1, xqT_cur, c_cur)
            xqT_cur, c_cur = _tail_layer(1, ys, bns)
            ys, bns = _main_layer(2, xqT_cur, c_cur)
            xqT_cur, c_cur = _tail_layer(2, ys, bns)

            # L3 + outputs
            ys, _ = _main_layer(3, xqT_cur, c_cur)
            for t in range(T):
                mzt = outpool.tile([P, OBINS], F32, tag="outr",
                                   name=f"mzt{t}")
                nc.vector.tensor_scalar(mzt[:], ys[t][:, 0:OBINS],
                                        float(OBINS - 1), 1.0,
                                        OP.mult, OP.add)
                nc.scalar.dma_start(mz_out[t * P:(t + 1) * P, :], mzt[:])
                iit = outpool.tile([P, OBINS], F32, tag="outr",
                                   name=f"iit{t}")
                nc.vector.tensor_scalar(iit[:], ys[t][:, OBINS:2 * OBINS],
                                        100.0, None, OP.mult)
                nc.scalar.dma_start(ii_out[t * P:(t + 1) * P, :], iit[:])

    nc.compile()
    return nc


def prepare_inputs(cfg, x, W0, W1, W2, W3):
    """Host-side sharding. wsh chunk order: (li, k, g, s); chunk content =
    WT[ic*128:(ic+1)*128, ob*512:(ob+1)*512] (p-major), ob = k*G+g,
    ic = r*S+s."""
    layers, total_ch, T = _plan(cfg)
    b_core = T * P
    Ws = [np.asarray(W0), np.asarray(W1), np.asarray(W2), np.asarray(W3)]
    WTs = []
    for L, W in zip(layers, Ws):
        WT = np.zeros((L["din"], L["dout"]), dtype=np.float32)
        WT[:, :L["dreal"]] = W.T
        WTs.append(WT)

    shards = [np.empty(total_ch * CH_ELS, dtype=np.float32)
              for _ in range(N_CORES)]
    for L in layers:
        li, G, S = L["li"], L["G"], L["S"]
        WT = WTs[li]
        for r in range(N_CORES):
            dst = shards[r]
            j = 0
            for k in range(L["n_ag"]):
                for g in range(G):
                    ob = k * G + g
                    for s in range(S):
                        ic = r * S + s
                        blk = WT[ic * P:(ic + 1) * P,
                                 ob * OBW:(ob + 1) * OBW]
                        off = (L["ch_base"] + j) * CH_ELS
                        dst[off:off + CH_ELS] = blk.reshape(-1)
                        j += 1
    x = np.asarray(x, dtype=np.float32)
    in_maps = []
    for r in range(N_CORES):
        in_maps.append(dict(
            xs=np.ascontiguousarray(x[r * b_core:(r + 1) * b_core]),
            wsh=shards[r],
        ))
    return in_maps


_NC_CACHE = {}


def _get_nc(cfg_key):
    if cfg_key not in _NC_CACHE:
        _NC_CACHE[cfg_key] = build(dict(cfg_key))
    return _NC_CACHE[cfg_key]


def run(cfg, x, W0, W1, W2, W3, trace=False):
    nc = _get_nc(tuple(sorted(cfg.items())))
    in_maps = prepare_inputs(cfg, x, W0, W1, W2, W3)
    res = run_bass_kernel_spmd(nc, in_maps, core_ids=list(range(N_CORES)),
                               trace=trace)
    mz = np.concatenate([res.results[r]["mz"] for r in range(N_CORES)], axis=0)
    ii = np.concatenate([res.results[r]["ii"] for r in range(N_CORES)], axis=0)
    return (mz, ii), res


def kernel(x, W0, W1, W2, W3, g0, b0, g1, b1, g2, b2):
    """Full-input entry point. g/b are identity (ones/zeros) in this
    problem's setup; LayerNorm affine is a no-op and is validated here."""
    for g in (g0, g1, g2):
        assert np.allclose(np.asarray(g), 1.0), "non-identity LN gain"
    for b in (b0, b1, b2):
        assert np.allclose(np.asarray(b), 0.0), "non-zero LN bias"
    (mz, ii), _ = run(FULL_CFG, x, W0, W1, W2, W3, trace=False)
    return (mz, ii)
